# revision 2
# baseline (speedup 1.0000x reference)
"""Trainium2 Bass kernel for nn_AttnMatching.

Reference computes:
    emb = emb_table[1:L+1]                      # [L, D]
    attn = einsum('ld,ntd->nlt', emb, self_attn)
    out  = einsum('nlt,t->nl', attn, value_w[0])

Reassociated (identical math, fp32):
    ctx[n, d] = sum_t value_w[t] * self_attn[n, t, d]    # [N, D]  (tiny)
    out[n, l] = sum_d ctx[n, d] * emb[l, d]              # [N, L]

Memory-bound: dominant traffic is streaming the 25.6 MB embedding table.
Sharding: vocab axis L split across 8 cores (6250 cols each),
self_attn/value_w replicated, no communication. Host-side marshalling
puts each tensor in its DMA-friendly layout:
  - emb shard pre-transposed to [D=128, Lsh] (contraction dim on
    partitions; 128 large per-partition descriptors per chunk).
  - self_attn transposed to [T, N*D] with value_w appended as the last
    column -> one [100, 2049] upload, 100 contiguous 8 KB descriptors.

Per-core Bass/Tile program:
  - attn+w upload on the sync (SP) HWDGE ring, then emb chunks
    alternating between the sync ring and the gpsimd (SWDGE) ring so
    descriptor-issue parallelizes; stores go on the scalar (ACT) ring.
  - 16 PE matmuls (lhsT=self_attn[n] [T,D], rhs=w [T,1]) accumulate
    ctxT columns -> PSUM [D, N] -> SBUF.
  - Main loop: lhsT=ctxT [D,16] stationary, rhs = emb chunks [D,<=512]
    streamed -> PSUM [16,<=512] -> DVE copy -> chunked store DMA.
"""

import numpy as np

L = 50000
D = 128
T = 100
N = 16
NCORES = 8
LSH = L // NCORES          # 6250 columns per core

DMA_CHUNK = 2048           # emb load granularity (1 MB per DMA)
MM_CHUNK = 512             # fp32 matmul moving-operand / PSUM bank limit

_cache = {}


def _chunks(total, step):
    return [(c0, min(c0 + step, total)) for c0 in range(0, total, step)]


def _build():
    import concourse.bacc as bacc
    import concourse.mybir as mybir
    import concourse.tile as tile

    nc = bacc.Bacc(
        "TRN2",
        target_bir_lowering=False,
        debug=False,
        enable_asserts=True,
        num_devices=NCORES,
    )

    embT = nc.dram_tensor("embT", [D, LSH], mybir.dt.float32, kind="ExternalInput").ap()
    attnw = nc.dram_tensor(
        "attnw", [T, N * D + 1], mybir.dt.float32, kind="ExternalInput"
    ).ap()
    out = nc.dram_tensor("out", [N, LSH], mybir.dt.float32, kind="ExternalOutput").ap()

    dma_chunks = _chunks(LSH, DMA_CHUNK)
    load_engines = [nc.sync, nc.gpsimd]

    with tile.TileContext(nc) as tc:
        with (
            tc.tile_pool(name="consts", bufs=1) as consts,
            tc.tile_pool(name="embp", bufs=len(dma_chunks)) as embp,
            tc.tile_pool(name="outp", bufs=3) as outp,
            tc.tile_pool(name="psc", bufs=1, space="PSUM") as psc,
            tc.tile_pool(name="pso", bufs=4, space="PSUM") as pso,
        ):
            attnw_tile = consts.tile([T, N * D + 1], mybir.dt.float32)
            nc.sync.dma_start(attnw_tile[:, :], attnw[:, :])

            emb_tiles = []
            for ci, (c0, c1) in enumerate(dma_chunks):
                et = embp.tile(
                    [D, c1 - c0], mybir.dt.float32, tag="emb", name=f"emb_{c0}"
                )
                load_engines[ci % 2].dma_start(et[:, :], embT[:, c0:c1])
                emb_tiles.append(et)

            # ctxT[d, n] = sum_t self_attn[n, t, d] * w[t]
            ps_ctx = psc.tile([D, N], mybir.dt.float32)
            for n in range(N):
                nc.tensor.matmul(
                    ps_ctx[:, n : n + 1],
                    lhsT=attnw_tile[:, n * D : (n + 1) * D],
                    rhs=attnw_tile[:, N * D : N * D + 1],
                    start=True,
                    stop=True,
                )
            ctxT = consts.tile([D, N], mybir.dt.float32)
            nc.vector.tensor_copy(ctxT[:, :], ps_ctx[:, :])

            # out[n, c0:c1] = ctxT.T @ embT[:, c0:c1]
            for ci, (c0, c1) in enumerate(dma_chunks):
                ot = outp.tile([N, c1 - c0], mybir.dt.float32, tag="out", name=f"out_{c0}")
                for s0, s1 in _chunks(c1 - c0, MM_CHUNK):
                    ps = pso.tile(
                        [N, s1 - s0], mybir.dt.float32, tag="pso", name=f"ps_{c0}_{s0}"
                    )
                    nc.tensor.matmul(
                        ps[:, :],
                        lhsT=ctxT[:, :],
                        rhs=emb_tiles[ci][:, s0:s1],
                        start=True,
                        stop=True,
                    )
                    nc.vector.tensor_copy(ot[:, s0:s1], ps[:, :])
                nc.scalar.dma_start(out[:, c0:c1], ot[:, :])

    nc.compile()
    return nc


def _get_nc():
    if "nc" not in _cache:
        _cache["nc"] = _build()
    return _cache["nc"]


def _make_in_maps(self_attn, emb_table, value_w):
    self_attn = np.asarray(self_attn, dtype=np.float32)
    value_w = np.asarray(value_w, dtype=np.float32)
    # [T, N*D + 1]: transposed self_attn with value_w as the last column
    attnw = np.empty((T, N * D + 1), dtype=np.float32)
    attnw[:, : N * D] = self_attn.transpose(1, 0, 2).reshape(T, N * D)
    attnw[:, N * D] = value_w[0]
    embT = np.asarray(emb_table, dtype=np.float32)[1 : L + 1].T  # [D, L]
    return [
        {
            "embT": np.ascontiguousarray(embT[:, k * LSH : (k + 1) * LSH]),
            "attnw": attnw,
            "wv_unused": None,
        }
        for k in range(NCORES)
    ]


def run(self_attn, emb_table, value_w, trace=False):
    from concourse.bass_utils import run_bass_kernel_spmd

    nc = _get_nc()
    in_maps = _make_in_maps(self_attn, emb_table, value_w)
    for m in in_maps:
        m.pop("wv_unused", None)
    res = run_bass_kernel_spmd(nc, in_maps, list(range(NCORES)), trace=trace)
    full = np.concatenate(
        [res.results[k]["out"] for k in range(NCORES)], axis=1
    ).astype(np.float32)
    return full, res


def kernel(self_attn, mat2, traj, emb_table, value_w):
    full, _ = run(self_attn, emb_table, value_w, trace=False)
    return full


# revision 9
# speedup vs baseline: 1.0286x; 1.0286x over previous
"""Trainium2 Bass kernel for nn_AttnMatching.

Reference computes:
    emb = emb_table[1:L+1]                      # [L, D]
    attn = einsum('ld,ntd->nlt', emb, self_attn)
    out  = einsum('nlt,t->nl', attn, value_w[0])

Reassociated (identical math, fp32):
    ctx[n, d] = sum_t value_w[t] * self_attn[n, t, d]    # [N, D]  (tiny)
    out[n, l] = sum_d ctx[n, d] * emb[l, d]              # [N, L]

Memory-bound: dominant traffic is streaming the 25.6 MB embedding table.
Sharding: vocab axis L split across 8 cores (6250 cols each),
self_attn/value_w replicated, no communication. Host-side marshalling
puts each tensor in its DMA-friendly layout:
  - emb shard pre-transposed to [D=128, Lsh] (contraction dim on
    partitions; 128 large per-partition descriptors per chunk).
  - self_attn transposed to [T, N*D] with value_w appended as the last
    column -> one [100, 2049] upload, 100 contiguous 8 KB descriptors.

Per-core Bass/Tile program:
  - attn+w upload on the sync (SP) HWDGE ring, then emb chunks
    alternating between the sync ring and the gpsimd (SWDGE) ring so
    descriptor-issue parallelizes; stores go on the scalar (ACT) ring.
  - 16 PE matmuls (lhsT=self_attn[n] [T,D], rhs=w [T,1]) accumulate
    ctxT columns -> PSUM [D, N] -> SBUF.
  - Main loop: lhsT=ctxT [D,16] stationary, rhs = emb chunks [D,<=512]
    streamed -> PSUM [16,<=512] -> DVE copy -> chunked store DMA.
"""

import os

import numpy as np

L = 50000
D = 128
T = 100
N = 16
NCORES = 8
LSH = L // NCORES          # 6250 columns per core

# knobs (env-overridable for A/B profiling)
DMA_CHUNK = int(os.environ.get("K_DMA_CHUNK", "2048"))  # emb load granularity
MM_CHUNK = 512             # matmul moving-operand / PSUM bank limit
MM_DT = os.environ.get("K_MM_DT", "float32r")  # matmul input dtype mode
NUM_DEVICES = int(os.environ.get("K_NUM_DEVICES", str(NCORES)))

_cache = {}


def _chunks(total, step):
    return [(c0, min(c0 + step, total)) for c0 in range(0, total, step)]


def _build():
    import concourse.bacc as bacc
    import concourse.mybir as mybir
    import concourse.tile as tile

    mm_dt = getattr(mybir.dt, MM_DT)

    nc = bacc.Bacc(
        "TRN2",
        target_bir_lowering=False,
        debug=False,
        enable_asserts=True,
        num_devices=NUM_DEVICES,
    )

    embT = nc.dram_tensor("embT", [D, LSH], mm_dt, kind="ExternalInput").ap()
    attnw = nc.dram_tensor(
        "attnw", [T, N * D + 1], mybir.dt.float32, kind="ExternalInput"
    ).ap()
    out = nc.dram_tensor("out", [N, LSH], mybir.dt.float32, kind="ExternalOutput").ap()

    dma_chunks = _chunks(LSH, DMA_CHUNK)
    load_engines = [nc.sync, nc.gpsimd]

    with tile.TileContext(nc) as tc:
        with (
            tc.tile_pool(name="consts", bufs=1) as consts,
            tc.tile_pool(name="embp", bufs=len(dma_chunks)) as embp,
            tc.tile_pool(name="outp", bufs=3) as outp,
            tc.tile_pool(name="psc", bufs=1, space="PSUM") as psc,
            tc.tile_pool(name="pso", bufs=4, space="PSUM") as pso,
        ):
            attnw_tile = consts.tile([T, N * D + 1], mybir.dt.float32)
            nc.sync.dma_start(attnw_tile[:, :], attnw[:, :])

            emb_tiles = []
            for ci, (c0, c1) in enumerate(dma_chunks):
                et = embp.tile(
                    [D, c1 - c0], mm_dt, tag="emb", name=f"emb_{c0}"
                )
                load_engines[ci % 2].dma_start(et[:, :], embT[:, c0:c1])
                emb_tiles.append(et)

            # ctxT[d, n] = sum_t self_attn[n, t, d] * w[t]
            ps_ctx = psc.tile([D, N], mybir.dt.float32)
            for n in range(N):
                nc.tensor.matmul(
                    ps_ctx[:, n : n + 1],
                    lhsT=attnw_tile[:, n * D : (n + 1) * D],
                    rhs=attnw_tile[:, N * D : N * D + 1],
                    start=True,
                    stop=True,
                )
            ctxT = consts.tile([D, N], mm_dt)
            nc.vector.tensor_copy(ctxT[:, :], ps_ctx[:, :])
            ctxT_mm = ctxT[:, :]

            # out[n, c0:c1] = ctxT.T @ embT[:, c0:c1]
            for ci, (c0, c1) in enumerate(dma_chunks):
                ot = outp.tile([N, c1 - c0], mybir.dt.float32, tag="out", name=f"out_{c0}")
                for s0, s1 in _chunks(c1 - c0, MM_CHUNK):
                    ps = pso.tile(
                        [N, s1 - s0], mybir.dt.float32, tag="pso", name=f"ps_{c0}_{s0}"
                    )
                    nc.tensor.matmul(
                        ps[:, :],
                        lhsT=ctxT_mm,
                        rhs=emb_tiles[ci][:, s0:s1],
                        start=True,
                        stop=True,
                    )
                    nc.vector.tensor_copy(ot[:, s0:s1], ps[:, :])
                nc.scalar.dma_start(out[:, c0:c1], ot[:, :])

    nc.compile()
    return nc


def _get_nc():
    if "nc" not in _cache:
        _cache["nc"] = _build()
    return _cache["nc"]


def _make_in_maps(self_attn, emb_table, value_w):
    self_attn = np.asarray(self_attn, dtype=np.float32)
    value_w = np.asarray(value_w, dtype=np.float32)
    # [T, N*D + 1]: transposed self_attn with value_w as the last column
    attnw = np.empty((T, N * D + 1), dtype=np.float32)
    attnw[:, : N * D] = self_attn.transpose(1, 0, 2).reshape(T, N * D)
    attnw[:, N * D] = value_w[0]
    embT = np.asarray(emb_table, dtype=np.float32)[1 : L + 1].T  # [D, L]
    return [
        {
            "embT": np.ascontiguousarray(embT[:, k * LSH : (k + 1) * LSH]),
            "attnw": attnw,
            "wv_unused": None,
        }
        for k in range(NCORES)
    ]


def run(self_attn, emb_table, value_w, trace=False):
    from concourse.bass_utils import run_bass_kernel_spmd

    nc = _get_nc()
    in_maps = _make_in_maps(self_attn, emb_table, value_w)
    for m in in_maps:
        m.pop("wv_unused", None)
    res = run_bass_kernel_spmd(nc, in_maps, list(range(NCORES)), trace=trace)
    full = np.concatenate(
        [res.results[k]["out"] for k in range(NCORES)], axis=1
    ).astype(np.float32)
    return full, res


def kernel(self_attn, mat2, traj, emb_table, value_w):
    full, _ = run(self_attn, emb_table, value_w, trace=False)
    return full


# revision 10
# speedup vs baseline: 1.0327x; 1.0040x over previous
"""Trainium2 Bass kernel for nn_AttnMatching.

Reference computes:
    emb = emb_table[1:L+1]                      # [L, D]
    attn = einsum('ld,ntd->nlt', emb, self_attn)
    out  = einsum('nlt,t->nl', attn, value_w[0])

Reassociated (identical math, fp32):
    ctx[n, d] = sum_t value_w[t] * self_attn[n, t, d]    # [N, D]  (tiny)
    out[n, l] = sum_d ctx[n, d] * emb[l, d]              # [N, L]

Memory-bound: dominant traffic is streaming the 25.6 MB embedding table.
Sharding: vocab axis L split across 8 cores (6250 cols each),
self_attn/value_w replicated, no communication. Host-side marshalling
puts each tensor in its DMA-friendly layout:
  - emb shard pre-transposed to [D=128, Lsh] (contraction dim on
    partitions; 128 large per-partition descriptors per chunk).
  - self_attn transposed to [T, N*D] with value_w appended as the last
    column -> one [100, 2049] upload, 100 contiguous 8 KB descriptors.

Per-core Bass/Tile program:
  - attn+w upload on the sync (SP) HWDGE ring, then emb chunks
    alternating between the sync ring and the gpsimd (SWDGE) ring so
    descriptor-issue parallelizes; stores go on the scalar (ACT) ring.
  - 16 PE matmuls (lhsT=self_attn[n] [T,D], rhs=w [T,1]) accumulate
    ctxT columns -> PSUM [D, N] -> SBUF.
  - Main loop: lhsT=ctxT [D,16] stationary, rhs = emb chunks [D,<=512]
    streamed -> PSUM [16,<=512] -> DVE copy -> chunked store DMA.
"""

import os

import numpy as np

L = 50000
D = 128
T = 100
N = 16
NCORES = 8
LSH = L // NCORES          # 6250 columns per core

# knobs (env-overridable for A/B profiling)
DMA_CHUNK = int(os.environ.get("K_DMA_CHUNK", "2048"))  # emb load granularity
MM_CHUNK = 512             # matmul moving-operand / PSUM bank limit
MM_DT = os.environ.get("K_MM_DT", "float32")  # matmul input dtype mode
NUM_DEVICES = int(os.environ.get("K_NUM_DEVICES", str(NCORES)))

_cache = {}


def _chunks(total, step):
    return [(c0, min(c0 + step, total)) for c0 in range(0, total, step)]


def _build():
    import concourse.bacc as bacc
    import concourse.mybir as mybir
    import concourse.tile as tile

    mm_dt = getattr(mybir.dt, MM_DT)

    nc = bacc.Bacc(
        "TRN2",
        target_bir_lowering=False,
        debug=False,
        enable_asserts=True,
        num_devices=NUM_DEVICES,
    )

    embT = nc.dram_tensor("embT", [D, LSH], mm_dt, kind="ExternalInput").ap()
    attnw = nc.dram_tensor(
        "attnw", [T, N * D + 1], mybir.dt.float32, kind="ExternalInput"
    ).ap()
    out = nc.dram_tensor("out", [N, LSH], mybir.dt.float32, kind="ExternalOutput").ap()

    dma_chunks = _chunks(LSH, DMA_CHUNK)
    load_engines = [nc.sync, nc.gpsimd]

    with tile.TileContext(nc) as tc:
        with (
            tc.tile_pool(name="consts", bufs=1) as consts,
            tc.tile_pool(name="embp", bufs=len(dma_chunks)) as embp,
            tc.tile_pool(name="outp", bufs=3) as outp,
            tc.tile_pool(name="psc", bufs=1, space="PSUM") as psc,
            tc.tile_pool(name="pso", bufs=4, space="PSUM") as pso,
        ):
            attnw_tile = consts.tile([T, N * D + 1], mybir.dt.float32)
            nc.sync.dma_start(attnw_tile[:, :], attnw[:, :])

            emb_tiles = []
            for ci, (c0, c1) in enumerate(dma_chunks):
                et = embp.tile(
                    [D, c1 - c0], mm_dt, tag="emb", name=f"emb_{c0}"
                )
                load_engines[ci % 2].dma_start(et[:, :], embT[:, c0:c1])
                emb_tiles.append(et)

            # ctxT[d, n] = sum_t self_attn[n, t, d] * w[t]
            ps_ctx = psc.tile([D, N], mybir.dt.float32)
            for n in range(N):
                nc.tensor.matmul(
                    ps_ctx[:, n : n + 1],
                    lhsT=attnw_tile[:, n * D : (n + 1) * D],
                    rhs=attnw_tile[:, N * D : N * D + 1],
                    start=True,
                    stop=True,
                )
            ctxT = consts.tile([D, N], mm_dt)
            nc.vector.tensor_copy(ctxT[:, :], ps_ctx[:, :])
            ctxT_mm = ctxT[:, :]

            # out[n, c0:c1] = ctxT.T @ embT[:, c0:c1]
            for ci, (c0, c1) in enumerate(dma_chunks):
                ot = outp.tile([N, c1 - c0], mybir.dt.float32, tag="out", name=f"out_{c0}")
                for s0, s1 in _chunks(c1 - c0, MM_CHUNK):
                    ps = pso.tile(
                        [N, s1 - s0], mybir.dt.float32, tag="pso", name=f"ps_{c0}_{s0}"
                    )
                    nc.tensor.matmul(
                        ps[:, :],
                        lhsT=ctxT_mm,
                        rhs=emb_tiles[ci][:, s0:s1],
                        start=True,
                        stop=True,
                    )
                    nc.vector.tensor_copy(ot[:, s0:s1], ps[:, :])
                nc.scalar.dma_start(out[:, c0:c1], ot[:, :])

    nc.compile()
    return nc


def _get_nc():
    if "nc" not in _cache:
        _cache["nc"] = _build()
    return _cache["nc"]


def _make_in_maps(self_attn, emb_table, value_w):
    self_attn = np.asarray(self_attn, dtype=np.float32)
    value_w = np.asarray(value_w, dtype=np.float32)
    # [T, N*D + 1]: transposed self_attn with value_w as the last column
    attnw = np.empty((T, N * D + 1), dtype=np.float32)
    attnw[:, : N * D] = self_attn.transpose(1, 0, 2).reshape(T, N * D)
    attnw[:, N * D] = value_w[0]
    embT = np.asarray(emb_table, dtype=np.float32)[1 : L + 1].T  # [D, L]
    return [
        {
            "embT": np.ascontiguousarray(embT[:, k * LSH : (k + 1) * LSH]),
            "attnw": attnw,
            "wv_unused": None,
        }
        for k in range(NCORES)
    ]


def run(self_attn, emb_table, value_w, trace=False):
    from concourse.bass_utils import run_bass_kernel_spmd

    nc = _get_nc()
    in_maps = _make_in_maps(self_attn, emb_table, value_w)
    for m in in_maps:
        m.pop("wv_unused", None)
    res = run_bass_kernel_spmd(nc, in_maps, list(range(NCORES)), trace=trace)
    full = np.concatenate(
        [res.results[k]["out"] for k in range(NCORES)], axis=1
    ).astype(np.float32)
    return full, res


def kernel(self_attn, mat2, traj, emb_table, value_w):
    full, _ = run(self_attn, emb_table, value_w, trace=False)
    return full
